# revision 6
# baseline (speedup 1.0000x reference)
"""Trainium2 Bass kernel for nn_HNetEnergyViaBoolWeights (v2).

Reference computation:
    a[n,e] = act[n, idx[e,0]],  b[n,e] = act[n, idx[e,1]]        (act is 0/1)
    code[n,e] = TEMP_TO_CODE[2a+b];  bact = one_hot(code)        (type-major)
    energies[n,c] = sum_k binarized_learned[c,k] * bact[n,k]
    out = energies - min(energies)

Multilinear identity over the learned one-hot planes P1..P4 (edge types
[0,2,3,5,9]; the actual edge code is never NULL so P0 drops out):

    match = P1 + (P3-P1)*a + (P2-P1)*b + (P4-P3-P2+P1)*ab

Device/host split (v2):
  * The constant plane folds to a host-side bias K[c] = sum_e P1[c,e].
  * The two LINEAR planes fold into NODE space on the host:
        sum_e Ca[c,e]*act[n,ia] + Cb[c,e]*act[n,ib] = Wnode[c,:] @ act[n,:]
    with Wnode = scatter-add of Ca/Cb over edge endpoints.  Wnode values
    are small ints (|W| <= ~50); they are split into two e4m3-exact fp8
    planes (lo + 16*hi) so the whole node term runs as 20 fp8 K=128
    matmul chunks per core against an fp8 activation slice.
  * Only the ab-product plane still needs per-edge data: one dma_gather
    per endpoint per 896-edge group, a DVE bitwise-AND (fp8 0/1 values
    AND bit-exactly; done in uint16 pairs for 2x mode), and 128-edge fp8
    matmul chunks accumulated in PSUM (DoubleRow: two chunks per
    instruction).
  * Gathers are issued as 896-index single-packet chunks round-robined
    over 4 SWDGE queues.  The gather is DESCRIPTOR-latency bound, not
    bandwidth bound (halving the row bytes does not speed it up), and
    queue concurrency is the lever: measured ~33-38us for the 2x6272
    row gathers vs ~116us for the monolithic single-queue form.

Sharding: edges are split across the 8 NeuronCores (6250 each) for the
gathered ab plane; nodes are split (1250 each) for the linear plane.
Every core computes a partial energy (64 cmp x 512 pts); the host sums
the 8 partials, adds K, and applies the global min subtraction (a
trivial (512,64) reduction).
"""

import numpy as np
import ml_dtypes

N_PTS = 512
N_NODES = 10000
N_EDGES = 50000
N_CMP = 64
N_CORES = 8
EDGES_PER_CORE = N_EDGES // N_CORES          # 6250
J_CHUNKS = -(-EDGES_PER_CORE // 128)         # 49 chunks of 128 edges
EDGES_PAD = J_CHUNKS * 128                   # 6272
# j-chunks per gather call: front-loaded sizes (<=7 chunks = 896 idxs,
# the single-packet sweet spot) with a small tail so the last AND+matmul
# group finishes quickly after the final gather lands.
G_SIZES = (7, 7, 7, 7, 6, 6, 5, 2, 2)        # sums to J_CHUNKS
N_GROUPS = len(G_SIZES)
NODES_PER_CORE = N_NODES // N_CORES          # 1250
V_CHUNKS = -(-NODES_PER_CORE // 128)         # 10 chunks of 128 nodes
NODES_PAD = V_CHUNKS * 128                   # 1280
F8 = ml_dtypes.float8_e4m3

_compiled = None


def _build_bass(repeats=1, loop_iters=0):
    """Build + compile the per-core Bass program (shared by all 8 cores).

    repeats>1 unrolls the whole body N times; loop_iters>0 wraps the body in
    a device-side For_i loop - both used only by the test harness to measure
    per-iteration device time by wall-clock differencing.
    """
    import concourse.mybir as mybir
    import concourse.tile as tile
    from concourse import bacc
    from concourse.library_config import mlp
    from contextlib import nullcontext

    dt = mybir.dt
    nc = bacc.Bacc("TRN2", target_bir_lowering=False, debug=False,
                   num_devices=N_CORES, num_swdge_queues=4)

    acts = nc.dram_tensor("acts", [N_NODES, N_PTS], dt.float8e4,
                          kind="ExternalInput")
    idx_a = nc.dram_tensor("idx_a", [128, EDGES_PAD // 16], dt.int16,
                           kind="ExternalInput")
    idx_b = nc.dram_tensor("idx_b", [128, EDGES_PAD // 16], dt.int16,
                           kind="ExternalInput")
    wp_ab = nc.dram_tensor("wp_ab", [128, J_CHUNKS, N_CMP], dt.float8e4,
                           kind="ExternalInput")
    wp_nd = nc.dram_tensor("wp_nd", [128, 2, V_CHUNKS, N_CMP], dt.float8e4,
                           kind="ExternalInput")
    acts_nd = nc.dram_tensor("acts_nd", [128, V_CHUNKS, N_PTS], dt.float8e4,
                             kind="ExternalInput")
    partial = nc.dram_tensor("partial", [N_CMP, N_PTS], dt.float32,
                             kind="ExternalOutput")

    with tile.TileContext(nc) as tc:
        with tc.tile_pool(name="sbuf", bufs=1) as pool, \
             tc.tile_pool(name="psum", bufs=1, space="PSUM") as psum_pool:
            nc.gpsimd.load_library(mlp)
            loop_cm = tc.For_i(0, loop_iters, 1) if loop_iters else nullcontext()
            with loop_cm:
                for rep in range(repeats):
                    idx_a_sb = pool.tile([128, EDGES_PAD // 16], dt.int16,
                                         tag="idx_a_sb")
                    idx_b_sb = pool.tile([128, EDGES_PAD // 16], dt.int16,
                                         tag="idx_b_sb")
                    wab_sb = pool.tile([128, J_CHUNKS, N_CMP], dt.float8e4,
                                       tag="wab_sb")
                    wnd_sb = pool.tile([128, 2, V_CHUNKS, N_CMP], dt.float8e4,
                                       tag="wnd_sb")
                    actnd_sb = pool.tile([128, V_CHUNKS, N_PTS], dt.float8e4,
                                         tag="actnd_sb")
                    nc.sync.dma_start(idx_a_sb[:], idx_a[:])
                    nc.sync.dma_start(idx_b_sb[:], idx_b[:])
                    nc.sync.dma_start(wab_sb[:], wp_ab[:])
                    nc.sync.dma_start(wnd_sb[:], wp_nd[:])
                    nc.sync.dma_start(actnd_sb[:], acts_nd[:])

                    ga = pool.tile([128, J_CHUNKS, N_PTS], dt.float8e4,
                                   tag="ga")
                    gb = pool.tile([128, J_CHUNKS, N_PTS], dt.float8e4,
                                   tag="gb")
                    gab = pool.tile([128, J_CHUNKS, N_PTS], dt.float8e4,
                                    tag="gab")

                    # single-packet gathers, a/b interleaved, 4-queue
                    # round-robin (see module docstring).
                    goff = [0]
                    for w in G_SIZES:
                        goff.append(goff[-1] + w)
                    q = 0
                    for g, w in enumerate(G_SIZES):
                        js = slice(goff[g], goff[g + 1])
                        cs = slice(goff[g] * 8, goff[g + 1] * 8)
                        for tsl, idxs in ((ga, idx_a_sb), (gb, idx_b_sb)):
                            nc.gpsimd.dma_gather(
                                tsl[:, js, :], acts[:], idxs[:, cs],
                                w * 128, w * 128, N_PTS,
                                single_packet=True, queue_num=q % 4)
                            q += 1

                    acc = psum_pool.tile([N_CMP, N_PTS], dt.float32,
                                         tag="acc")

                    # node-space linear term: two exact fp8 planes
                    # (lo + 16*hi); first into the PSUM accumulation
                    # group, runs while the gathers drain.
                    first = True
                    for h in range(2):
                        for v in range(V_CHUNKS):
                            nc.tensor.matmul(
                                acc[:], wnd_sb[:, h, v, :],
                                actnd_sb[:, v, :],
                                start=first, stop=False)
                            first = False

                    # ab plane: AND + DoubleRow matmuls per gather group,
                    # pipelined behind the gathers.
                    pair = 0
                    for g in range(N_GROUPS):
                        js = slice(goff[g], goff[g + 1])
                        nc.vector.tensor_tensor(
                            gab[:, js, :].bitcast(dt.uint16),
                            ga[:, js, :].bitcast(dt.uint16),
                            gb[:, js, :].bitcast(dt.uint16),
                            op=mybir.AluOpType.bitwise_and)
                        j_done = goff[g + 1]
                        while (pair + 1) * 2 <= j_done:
                            j0 = pair * 2
                            nc.tensor.matmul(
                                acc[:], wab_sb[:, j0:j0 + 2, :],
                                gab[:, j0:j0 + 2, :],
                                start=False, stop=False,
                                perf_mode=mybir.MatmulPerfMode.DoubleRow)
                            pair += 1
                        if j_done == J_CHUNKS:
                            j = J_CHUNKS - 1   # odd chunk count: one single
                            nc.tensor.matmul(
                                acc[:], wab_sb[:, j, :],
                                gab[:, j, :], start=False, stop=True)

                    out_sb = pool.tile([N_CMP, N_PTS], dt.float32,
                                       tag="out_sb")
                    nc.vector.tensor_copy(out_sb[:], acc[:])
                    nc.sync.dma_start(partial[:], out_sb[:])

    nc.compile()
    return nc


def _get_compiled():
    global _compiled
    if _compiled is None:
        _compiled = _build_bass()
    return _compiled


def _wrap_idx(idx_slice):
    """int32 edge-endpoint slice -> (128, EDGES_PAD//16) int16 wrapped layout
    (index i at partition i%16, column i//16; 16-row block replicated 8x)."""
    pad = np.zeros(EDGES_PAD, np.int16)
    pad[:idx_slice.shape[0]] = idx_slice.astype(np.int16)
    arr16 = pad.reshape(EDGES_PAD // 16, 16).T.copy()
    return np.tile(arr16, (8, 1))


def prepare_in_maps(node_activations, binarized_learned, edge_endnode_idx,
                    sort_edges=True):
    """Host-side sharding/relayout. Returns (in_maps, K_bias).

    sort_edges: reorder each core's edge slots by (ia >> 8, ib) - the ab
    plane is symmetric in the two gathered operands and slot order is
    free (coefficients follow), so sorting costs nothing and gives the
    a-gather 128KB-blocked row locality and the b-gather long sorted
    runs within each block."""
    act = np.asarray(node_activations)
    W = np.asarray(binarized_learned)
    idx = np.asarray(edge_endnode_idx)

    # fp8 node-activation table, transposed: row v = act[:, v] over 512 pts
    acts_t = np.ascontiguousarray(act.T).astype(np.float32).astype(F8)

    # weight planes over EDGE_TYPES=[0,2,3,5,9]; multilinear coefficients
    P = W.reshape(N_CMP, 5, N_EDGES)
    P1, P2, P3, P4 = P[:, 1], P[:, 2], P[:, 3], P[:, 4]
    K_bias = P1.sum(axis=1, dtype=np.float64).astype(np.float32)  # (64,)
    coeff_a = P3 - P1
    coeff_b = P2 - P1
    coeff_ab = P4 - P3 - P2 + P1

    # node-space fold of the two linear planes: Wnode (N_NODES, N_CMP)
    Wnode = np.zeros((N_NODES, N_CMP), np.float32)
    np.add.at(Wnode, idx[:, 0], coeff_a.T)
    np.add.at(Wnode, idx[:, 1], coeff_b.T)

    # transposed activations for the node matmul: actnd_full[v, n] = act[n, v]
    actnd_full = np.ascontiguousarray(act.T).astype(np.float32)

    in_maps = []
    for s in range(N_CORES):
        sl = slice(s * EDGES_PER_CORE, (s + 1) * EDGES_PER_CORE)
        ia = idx[sl, 0]
        ib = idx[sl, 1]
        cab = coeff_ab[:, sl]
        if sort_edges:
            perm = np.lexsort((ib, ia >> 8))
            ia, ib, cab = ia[perm], ib[perm], cab[:, perm]
        cpad = np.zeros((N_CMP, EDGES_PAD), np.float32)
        cpad[:, :EDGES_PER_CORE] = cab
        # lhsT layout: wab[p, j, c] = coeff_ab[c, j*128 + p]
        wab = np.ascontiguousarray(
            cpad.reshape(N_CMP, J_CHUNKS, 128).transpose(2, 1, 0)
        ).astype(F8)

        vsl = slice(s * NODES_PER_CORE, (s + 1) * NODES_PER_CORE)
        wnd_pad = np.zeros((NODES_PAD, N_CMP), np.float32)
        wnd_pad[:NODES_PER_CORE] = Wnode[vsl]
        # split into two exact fp8 planes: W = lo + 16*hi, lo in [-8,8],
        # 16*hi in {0,+-16,+-32,+-48,...} - all e4m3-exact for |W| <= ~400
        hi = np.round(wnd_pad / 16.0)
        lo = wnd_pad - 16.0 * hi
        assert np.abs(hi).max() * 16 <= 448 and np.abs(lo).max() <= 8
        # lhsT layout [128, 2, V_CHUNKS, N_CMP]: [p,h,v,c] = plane_h[v*128+p, c]
        wnd = np.ascontiguousarray(
            np.stack([lo, 16.0 * hi], axis=0)
            .reshape(2, V_CHUNKS, 128, N_CMP).transpose(2, 0, 1, 3)
        ).astype(F8)

        and_pad = np.zeros((NODES_PAD, N_PTS), np.float32)
        and_pad[:NODES_PER_CORE] = actnd_full[vsl]
        actnd = np.ascontiguousarray(
            and_pad.reshape(V_CHUNKS, 128, N_PTS).transpose(1, 0, 2)
        ).astype(F8)

        in_maps.append({
            "acts": acts_t,
            "idx_a": _wrap_idx(ia),
            "idx_b": _wrap_idx(ib),
            "wp_ab": wab,
            "wp_nd": wnd,
            "acts_nd": actnd,
        })
    return in_maps, K_bias


def postprocess(results, K_bias):
    """Sum per-core partials, add bias, subtract global min."""
    total = np.zeros((N_CMP, N_PTS), np.float32)
    for r in results:
        total += r["partial"]
    energies = total + K_bias[:, None]
    out = energies.T - energies.min()
    return np.ascontiguousarray(out.astype(np.float32))


def kernel(node_activations, binarized_learned, edge_endnode_idx,
           _bass_kwargs=None):
    from concourse.bass_utils import run_bass_kernel_spmd

    nc = _get_compiled()
    in_maps, K_bias = prepare_in_maps(
        node_activations, binarized_learned, edge_endnode_idx)
    res = run_bass_kernel_spmd(nc, in_maps, core_ids=list(range(N_CORES)),
                               **(_bass_kwargs or {}))
    out = postprocess(res.results, K_bias)
    kernel.last_results = res
    return out


# revision 7
# speedup vs baseline: 1.0100x; 1.0100x over previous
"""Trainium2 Bass kernel for nn_HNetEnergyViaBoolWeights (v2).

Reference computation:
    a[n,e] = act[n, idx[e,0]],  b[n,e] = act[n, idx[e,1]]        (act is 0/1)
    code[n,e] = TEMP_TO_CODE[2a+b];  bact = one_hot(code)        (type-major)
    energies[n,c] = sum_k binarized_learned[c,k] * bact[n,k]
    out = energies - min(energies)

Multilinear identity over the learned one-hot planes P1..P4 (edge types
[0,2,3,5,9]; the actual edge code is never NULL so P0 drops out):

    match = P1 + (P3-P1)*a + (P2-P1)*b + (P4-P3-P2+P1)*ab

Device/host split (v2):
  * The constant plane folds to a host-side bias K[c] = sum_e P1[c,e].
  * The two LINEAR planes fold into NODE space on the host:
        sum_e Ca[c,e]*act[n,ia] + Cb[c,e]*act[n,ib] = Wnode[c,:] @ act[n,:]
    with Wnode = scatter-add of Ca/Cb over edge endpoints.  Wnode values
    are small ints (|W| <= ~50); they are split into two e4m3-exact fp8
    planes (lo + 16*hi) so the whole node term runs as 20 fp8 K=128
    matmul chunks per core against an fp8 activation slice.
  * Only the ab-product plane still needs per-edge data: one dma_gather
    per endpoint per 896-edge group, a DVE bitwise-AND (fp8 0/1 values
    AND bit-exactly; done in uint16 pairs for 2x mode), and 128-edge fp8
    matmul chunks accumulated in PSUM (DoubleRow: two chunks per
    instruction).
  * Gathers are issued as 896-index single-packet chunks round-robined
    over 4 SWDGE queues.  The gather is DESCRIPTOR-latency bound, not
    bandwidth bound (halving the row bytes does not speed it up), and
    queue concurrency is the lever: measured ~33-38us for the 2x6272
    row gathers vs ~116us for the monolithic single-queue form.

Sharding: edges are split across the 8 NeuronCores (6250 each) for the
gathered ab plane; nodes are split (1250 each) for the linear plane.
Every core computes a partial energy (64 cmp x 512 pts); the host sums
the 8 partials, adds K, and applies the global min subtraction (a
trivial (512,64) reduction).
"""

import numpy as np
import ml_dtypes

N_PTS = 512
N_NODES = 10000
N_EDGES = 50000
N_CMP = 64
N_CORES = 8
EDGES_PER_CORE = N_EDGES // N_CORES          # 6250
J_CHUNKS = -(-EDGES_PER_CORE // 128)         # 49 chunks of 128 edges
EDGES_PAD = J_CHUNKS * 128                   # 6272
# j-chunks per gather call: front-loaded sizes (<=7 chunks = 896 idxs,
# the single-packet sweet spot) with a small tail so the last AND+matmul
# group finishes quickly after the final gather lands.
G_SIZES = (7, 7, 7, 7, 6, 6, 5, 2, 2)        # sums to J_CHUNKS
N_GROUPS = len(G_SIZES)
NODES_PER_CORE = N_NODES // N_CORES          # 1250
V_CHUNKS = -(-NODES_PER_CORE // 128)         # 10 chunks of 128 nodes
NODES_PAD = V_CHUNKS * 128                   # 1280
F8 = ml_dtypes.float8_e4m3

_compiled = None


def _build_bass(repeats=1, loop_iters=0):
    """Build + compile the per-core Bass program (shared by all 8 cores).

    repeats>1 unrolls the whole body N times; loop_iters>0 wraps the body in
    a device-side For_i loop - both used only by the test harness to measure
    per-iteration device time by wall-clock differencing.
    """
    import concourse.mybir as mybir
    import concourse.tile as tile
    from concourse import bacc
    from concourse.library_config import mlp
    from contextlib import nullcontext

    dt = mybir.dt
    nc = bacc.Bacc("TRN2", target_bir_lowering=False, debug=False,
                   num_devices=N_CORES, num_swdge_queues=4)

    acts = nc.dram_tensor("acts", [N_NODES, N_PTS], dt.float8e4,
                          kind="ExternalInput")
    idx_a = nc.dram_tensor("idx_a", [128, EDGES_PAD // 16], dt.int16,
                           kind="ExternalInput")
    idx_b = nc.dram_tensor("idx_b", [128, EDGES_PAD // 16], dt.int16,
                           kind="ExternalInput")
    wp_ab = nc.dram_tensor("wp_ab", [128, J_CHUNKS, N_CMP], dt.float8e4,
                           kind="ExternalInput")
    wp_nd = nc.dram_tensor("wp_nd", [128, 2, V_CHUNKS, N_CMP], dt.float8e4,
                           kind="ExternalInput")
    acts_nd = nc.dram_tensor("acts_nd", [128, V_CHUNKS, N_PTS], dt.float8e4,
                             kind="ExternalInput")
    partial = nc.dram_tensor("partial", [N_CMP, N_PTS], dt.float32,
                             kind="ExternalOutput")

    with tile.TileContext(nc) as tc:
        with tc.tile_pool(name="sbuf", bufs=1) as pool, \
             tc.tile_pool(name="psum", bufs=1, space="PSUM") as psum_pool:
            nc.gpsimd.load_library(mlp)
            loop_cm = tc.For_i(0, loop_iters, 1) if loop_iters else nullcontext()
            with loop_cm:
                for rep in range(repeats):
                    idx_a_sb = pool.tile([128, EDGES_PAD // 16], dt.int16,
                                         tag="idx_a_sb")
                    idx_b_sb = pool.tile([128, EDGES_PAD // 16], dt.int16,
                                         tag="idx_b_sb")
                    wab_sb = pool.tile([128, J_CHUNKS, N_CMP], dt.float8e4,
                                       tag="wab_sb")
                    wnd_sb = pool.tile([128, 2, V_CHUNKS, N_CMP], dt.float8e4,
                                       tag="wnd_sb")
                    actnd_sb = pool.tile([128, V_CHUNKS, N_PTS], dt.float8e4,
                                         tag="actnd_sb")
                    nc.sync.dma_start(idx_a_sb[:], idx_a[:])
                    nc.sync.dma_start(idx_b_sb[:], idx_b[:])
                    nc.sync.dma_start(wab_sb[:], wp_ab[:])
                    nc.sync.dma_start(wnd_sb[:], wp_nd[:])
                    nc.sync.dma_start(actnd_sb[:], acts_nd[:])

                    ga = pool.tile([128, J_CHUNKS, N_PTS], dt.float8e4,
                                   tag="ga")
                    gb = pool.tile([128, J_CHUNKS, N_PTS], dt.float8e4,
                                   tag="gb")
                    gab = pool.tile([128, J_CHUNKS, N_PTS], dt.float8e4,
                                    tag="gab")

                    # single-packet gathers, a/b interleaved, 4-queue
                    # round-robin (see module docstring).
                    goff = [0]
                    for w in G_SIZES:
                        goff.append(goff[-1] + w)
                    q = 0
                    for g, w in enumerate(G_SIZES):
                        js = slice(goff[g], goff[g + 1])
                        cs = slice(goff[g] * 8, goff[g + 1] * 8)
                        for tsl, idxs in ((ga, idx_a_sb), (gb, idx_b_sb)):
                            nc.gpsimd.dma_gather(
                                tsl[:, js, :], acts[:], idxs[:, cs],
                                w * 128, w * 128, N_PTS,
                                single_packet=True, queue_num=q % 4)
                            q += 1

                    acc = psum_pool.tile([N_CMP, N_PTS], dt.float32,
                                         tag="acc")

                    # node-space linear term: two exact fp8 planes
                    # (lo + 16*hi); first into the PSUM accumulation
                    # group, runs while the gathers drain.
                    first = True
                    for h in range(2):
                        for v in range(V_CHUNKS):
                            nc.tensor.matmul(
                                acc[:], wnd_sb[:, h, v, :],
                                actnd_sb[:, v, :],
                                start=first, stop=False)
                            first = False

                    # ab plane: AND + DoubleRow matmuls per gather group,
                    # pipelined behind the gathers.
                    pair = 0
                    for g in range(N_GROUPS):
                        js = slice(goff[g], goff[g + 1])
                        nc.vector.tensor_tensor(
                            gab[:, js, :].bitcast(dt.uint16),
                            ga[:, js, :].bitcast(dt.uint16),
                            gb[:, js, :].bitcast(dt.uint16),
                            op=mybir.AluOpType.bitwise_and)
                        j_done = goff[g + 1]
                        while (pair + 1) * 2 <= j_done:
                            j0 = pair * 2
                            nc.tensor.matmul(
                                acc[:], wab_sb[:, j0:j0 + 2, :],
                                gab[:, j0:j0 + 2, :],
                                start=False, stop=False,
                                perf_mode=mybir.MatmulPerfMode.DoubleRow)
                            pair += 1
                        if j_done == J_CHUNKS:
                            j = J_CHUNKS - 1   # odd chunk count: one single
                            nc.tensor.matmul(
                                acc[:], wab_sb[:, j, :],
                                gab[:, j, :], start=False, stop=True)

                    out_sb = pool.tile([N_CMP, N_PTS], dt.float32,
                                       tag="out_sb")
                    nc.vector.tensor_copy(out_sb[:], acc[:])
                    nc.sync.dma_start(partial[:], out_sb[:])

    nc.compile()
    return nc


def _get_compiled():
    global _compiled
    if _compiled is None:
        _compiled = _build_bass()
    return _compiled


def _wrap_idx(idx_slice):
    """int32 edge-endpoint slice -> (128, EDGES_PAD//16) int16 wrapped layout
    (index i at partition i%16, column i//16; 16-row block replicated 8x)."""
    pad = np.zeros(EDGES_PAD, np.int16)
    pad[:idx_slice.shape[0]] = idx_slice.astype(np.int16)
    arr16 = pad.reshape(EDGES_PAD // 16, 16).T.copy()
    return np.tile(arr16, (8, 1))


def prepare_in_maps(node_activations, binarized_learned, edge_endnode_idx,
                    sort_edges=True):
    """Host-side sharding/relayout. Returns (in_maps, K_bias).

    sort_edges: the ab plane is symmetric in the two gathered operands,
    so both the slot order and the per-edge operand orientation are free
    (coefficients follow).  Orient a=min(u,v), b=max(u,v) and sort slots
    by (a >> 9, b): the a-gather walks 256KB row blocks and the b-gather
    gets sorted runs within each block - measured ~5-20% faster than
    unsorted via DRAM row-buffer locality, at zero device cost."""
    act = np.asarray(node_activations)
    W = np.asarray(binarized_learned)
    idx = np.asarray(edge_endnode_idx)

    # fp8 node-activation table, transposed: row v = act[:, v] over 512 pts
    acts_t = np.ascontiguousarray(act.T).astype(np.float32).astype(F8)

    # weight planes over EDGE_TYPES=[0,2,3,5,9]; multilinear coefficients
    P = W.reshape(N_CMP, 5, N_EDGES)
    P1, P2, P3, P4 = P[:, 1], P[:, 2], P[:, 3], P[:, 4]
    K_bias = P1.sum(axis=1, dtype=np.float64).astype(np.float32)  # (64,)
    coeff_a = P3 - P1
    coeff_b = P2 - P1
    coeff_ab = P4 - P3 - P2 + P1

    # node-space fold of the two linear planes: Wnode (N_NODES, N_CMP)
    Wnode = np.zeros((N_NODES, N_CMP), np.float32)
    np.add.at(Wnode, idx[:, 0], coeff_a.T)
    np.add.at(Wnode, idx[:, 1], coeff_b.T)

    # transposed activations for the node matmul: actnd_full[v, n] = act[n, v]
    actnd_full = np.ascontiguousarray(act.T).astype(np.float32)

    in_maps = []
    for s in range(N_CORES):
        sl = slice(s * EDGES_PER_CORE, (s + 1) * EDGES_PER_CORE)
        ia = idx[sl, 0]
        ib = idx[sl, 1]
        cab = coeff_ab[:, sl]
        if sort_edges:
            lo = np.minimum(ia, ib)
            hi = np.maximum(ia, ib)
            perm = np.lexsort((hi, lo >> 9))
            ia, ib, cab = lo[perm], hi[perm], cab[:, perm]
        cpad = np.zeros((N_CMP, EDGES_PAD), np.float32)
        cpad[:, :EDGES_PER_CORE] = cab
        # lhsT layout: wab[p, j, c] = coeff_ab[c, j*128 + p]
        wab = np.ascontiguousarray(
            cpad.reshape(N_CMP, J_CHUNKS, 128).transpose(2, 1, 0)
        ).astype(F8)

        vsl = slice(s * NODES_PER_CORE, (s + 1) * NODES_PER_CORE)
        wnd_pad = np.zeros((NODES_PAD, N_CMP), np.float32)
        wnd_pad[:NODES_PER_CORE] = Wnode[vsl]
        # split into two exact fp8 planes: W = lo + 16*hi, lo in [-8,8],
        # 16*hi in {0,+-16,+-32,+-48,...} - all e4m3-exact for |W| <= ~400
        hi = np.round(wnd_pad / 16.0)
        lo = wnd_pad - 16.0 * hi
        assert np.abs(hi).max() * 16 <= 448 and np.abs(lo).max() <= 8
        # lhsT layout [128, 2, V_CHUNKS, N_CMP]: [p,h,v,c] = plane_h[v*128+p, c]
        wnd = np.ascontiguousarray(
            np.stack([lo, 16.0 * hi], axis=0)
            .reshape(2, V_CHUNKS, 128, N_CMP).transpose(2, 0, 1, 3)
        ).astype(F8)

        and_pad = np.zeros((NODES_PAD, N_PTS), np.float32)
        and_pad[:NODES_PER_CORE] = actnd_full[vsl]
        actnd = np.ascontiguousarray(
            and_pad.reshape(V_CHUNKS, 128, N_PTS).transpose(1, 0, 2)
        ).astype(F8)

        in_maps.append({
            "acts": acts_t,
            "idx_a": _wrap_idx(ia),
            "idx_b": _wrap_idx(ib),
            "wp_ab": wab,
            "wp_nd": wnd,
            "acts_nd": actnd,
        })
    return in_maps, K_bias


def postprocess(results, K_bias):
    """Sum per-core partials, add bias, subtract global min."""
    total = np.zeros((N_CMP, N_PTS), np.float32)
    for r in results:
        total += r["partial"]
    energies = total + K_bias[:, None]
    out = energies.T - energies.min()
    return np.ascontiguousarray(out.astype(np.float32))


def kernel(node_activations, binarized_learned, edge_endnode_idx,
           _bass_kwargs=None):
    from concourse.bass_utils import run_bass_kernel_spmd

    nc = _get_compiled()
    in_maps, K_bias = prepare_in_maps(
        node_activations, binarized_learned, edge_endnode_idx)
    res = run_bass_kernel_spmd(nc, in_maps, core_ids=list(range(N_CORES)),
                               **(_bass_kwargs or {}))
    out = postprocess(res.results, K_bias)
    kernel.last_results = res
    return out


# revision 9
# speedup vs baseline: 1.0155x; 1.0055x over previous
"""Trainium2 Bass kernel for nn_HNetEnergyViaBoolWeights (v2).

Reference computation:
    a[n,e] = act[n, idx[e,0]],  b[n,e] = act[n, idx[e,1]]        (act is 0/1)
    code[n,e] = TEMP_TO_CODE[2a+b];  bact = one_hot(code)        (type-major)
    energies[n,c] = sum_k binarized_learned[c,k] * bact[n,k]
    out = energies - min(energies)

Multilinear identity over the learned one-hot planes P1..P4 (edge types
[0,2,3,5,9]; the actual edge code is never NULL so P0 drops out):

    match = P1 + (P3-P1)*a + (P2-P1)*b + (P4-P3-P2+P1)*ab

Device/host split (v2):
  * The constant plane folds to a host-side bias K[c] = sum_e P1[c,e].
  * The two LINEAR planes fold into NODE space on the host:
        sum_e Ca[c,e]*act[n,ia] + Cb[c,e]*act[n,ib] = Wnode[c,:] @ act[n,:]
    with Wnode = scatter-add of Ca/Cb over edge endpoints.  Wnode values
    are small ints (|W| <= ~50); they are split into two e4m3-exact fp8
    planes (lo + 16*hi) so the whole node term runs as 20 fp8 K=128
    matmul chunks per core against an fp8 activation slice.
  * Only the ab-product plane still needs per-edge data: one dma_gather
    per endpoint per 896-edge group, a DVE bitwise-AND (fp8 0/1 values
    AND bit-exactly; done in uint16 pairs for 2x mode), and 128-edge fp8
    matmul chunks accumulated in PSUM (DoubleRow: two chunks per
    instruction).
  * Gathers are issued as 896-index single-packet chunks round-robined
    over 4 SWDGE queues.  The gather is DESCRIPTOR-latency bound, not
    bandwidth bound (halving the row bytes does not speed it up), and
    queue concurrency is the lever: measured ~33-38us for the 2x6272
    row gathers vs ~116us for the monolithic single-queue form.

Sharding: edges are split across the 8 NeuronCores (6250 each) for the
gathered ab plane; nodes are split (1250 each) for the linear plane.
Every core computes a partial energy (64 cmp x 512 pts); the host sums
the 8 partials, adds K, and applies the global min subtraction (a
trivial (512,64) reduction).
"""

import numpy as np
import ml_dtypes

N_PTS = 512
N_NODES = 10000
N_EDGES = 50000
N_CMP = 64
N_CORES = 8
EDGES_PER_CORE = N_EDGES // N_CORES          # 6250
J_CHUNKS = -(-EDGES_PER_CORE // 128)         # 49 chunks of 128 edges
EDGES_PAD = J_CHUNKS * 128                   # 6272
# j-chunks per gather call: front-loaded sizes (<=7 chunks = 896 idxs,
# the single-packet sweet spot) with a small tail so the last AND+matmul
# group finishes quickly after the final gather lands.
G_SIZES = (7, 7, 7, 7, 6, 6, 5, 2, 2)        # sums to J_CHUNKS
N_GROUPS = len(G_SIZES)
NODES_PER_CORE = N_NODES // N_CORES          # 1250
V_CHUNKS = -(-NODES_PER_CORE // 128)         # 10 chunks of 128 nodes
NODES_PAD = V_CHUNKS * 128                   # 1280
F8 = ml_dtypes.float8_e4m3

_compiled = None


def _build_bass(repeats=1, loop_iters=0):
    """Build + compile the per-core Bass program (shared by all 8 cores).

    repeats>1 unrolls the whole body N times; loop_iters>0 wraps the body in
    a device-side For_i loop - both used only by the test harness to measure
    per-iteration device time by wall-clock differencing.
    """
    import concourse.mybir as mybir
    import concourse.tile as tile
    from concourse import bacc
    from concourse.library_config import mlp
    from contextlib import nullcontext

    dt = mybir.dt
    nc = bacc.Bacc("TRN2", target_bir_lowering=False, debug=False,
                   num_devices=N_CORES, num_swdge_queues=4)

    acts = nc.dram_tensor("acts", [N_NODES, N_PTS], dt.float8e4,
                          kind="ExternalInput")
    acts_b = nc.dram_tensor("acts_b", [N_NODES, N_PTS], dt.float8e4,
                            kind="ExternalInput")
    idx_a = nc.dram_tensor("idx_a", [128, EDGES_PAD // 16], dt.int16,
                           kind="ExternalInput")
    idx_b = nc.dram_tensor("idx_b", [128, EDGES_PAD // 16], dt.int16,
                           kind="ExternalInput")
    wp_ab = nc.dram_tensor("wp_ab", [128, J_CHUNKS, N_CMP], dt.float8e4,
                           kind="ExternalInput")
    wp_nd = nc.dram_tensor("wp_nd", [128, 2, V_CHUNKS, N_CMP], dt.float8e4,
                           kind="ExternalInput")
    acts_nd = nc.dram_tensor("acts_nd", [128, V_CHUNKS, N_PTS], dt.float8e4,
                             kind="ExternalInput")
    partial = nc.dram_tensor("partial", [N_CMP, N_PTS], dt.float32,
                             kind="ExternalOutput")

    with tile.TileContext(nc) as tc:
        with tc.tile_pool(name="sbuf", bufs=1) as pool, \
             tc.tile_pool(name="psum", bufs=1, space="PSUM") as psum_pool:
            nc.gpsimd.load_library(mlp)
            loop_cm = tc.For_i(0, loop_iters, 1) if loop_iters else nullcontext()
            with loop_cm:
                for rep in range(repeats):
                    idx_a_sb = pool.tile([128, EDGES_PAD // 16], dt.int16,
                                         tag="idx_a_sb")
                    idx_b_sb = pool.tile([128, EDGES_PAD // 16], dt.int16,
                                         tag="idx_b_sb")
                    wab_sb = pool.tile([128, J_CHUNKS, N_CMP], dt.float8e4,
                                       tag="wab_sb")
                    wnd_sb = pool.tile([128, 2, V_CHUNKS, N_CMP], dt.float8e4,
                                       tag="wnd_sb")
                    actnd_sb = pool.tile([128, V_CHUNKS, N_PTS], dt.float8e4,
                                         tag="actnd_sb")
                    nc.sync.dma_start(idx_a_sb[:], idx_a[:])
                    nc.sync.dma_start(idx_b_sb[:], idx_b[:])
                    nc.sync.dma_start(wab_sb[:], wp_ab[:])
                    nc.sync.dma_start(wnd_sb[:], wp_nd[:])
                    nc.sync.dma_start(actnd_sb[:], acts_nd[:])

                    ga = pool.tile([128, J_CHUNKS, N_PTS], dt.float8e4,
                                   tag="ga")
                    gb = pool.tile([128, J_CHUNKS, N_PTS], dt.float8e4,
                                   tag="gb")
                    gab = pool.tile([128, J_CHUNKS, N_PTS], dt.float8e4,
                                    tag="gab")

                    # single-packet gathers, a/b interleaved, 4-queue
                    # round-robin (see module docstring).
                    goff = [0]
                    for w in G_SIZES:
                        goff.append(goff[-1] + w)
                    q = 0
                    for g, w in enumerate(G_SIZES):
                        js = slice(goff[g], goff[g + 1])
                        cs = slice(goff[g] * 8, goff[g + 1] * 8)
                        for tsl, src, idxs in ((ga, acts, idx_a_sb),
                                               (gb, acts_b, idx_b_sb)):
                            nc.gpsimd.dma_gather(
                                tsl[:, js, :], src[:], idxs[:, cs],
                                w * 128, w * 128, N_PTS,
                                single_packet=True, queue_num=q % 4)
                            q += 1

                    acc = psum_pool.tile([N_CMP, N_PTS], dt.float32,
                                         tag="acc")

                    # node-space linear term: two exact fp8 planes
                    # (lo + 16*hi); first into the PSUM accumulation
                    # group, runs while the gathers drain.
                    first = True
                    for h in range(2):
                        for v in range(V_CHUNKS):
                            nc.tensor.matmul(
                                acc[:], wnd_sb[:, h, v, :],
                                actnd_sb[:, v, :],
                                start=first, stop=False)
                            first = False

                    # ab plane: AND + DoubleRow matmuls per gather group,
                    # pipelined behind the gathers.
                    pair = 0
                    for g in range(N_GROUPS):
                        js = slice(goff[g], goff[g + 1])
                        nc.vector.tensor_tensor(
                            gab[:, js, :].bitcast(dt.uint16),
                            ga[:, js, :].bitcast(dt.uint16),
                            gb[:, js, :].bitcast(dt.uint16),
                            op=mybir.AluOpType.bitwise_and)
                        j_done = goff[g + 1]
                        while (pair + 1) * 2 <= j_done:
                            j0 = pair * 2
                            nc.tensor.matmul(
                                acc[:], wab_sb[:, j0:j0 + 2, :],
                                gab[:, j0:j0 + 2, :],
                                start=False, stop=False,
                                perf_mode=mybir.MatmulPerfMode.DoubleRow)
                            pair += 1
                        if j_done == J_CHUNKS:
                            j = J_CHUNKS - 1   # odd chunk count: one single
                            nc.tensor.matmul(
                                acc[:], wab_sb[:, j, :],
                                gab[:, j, :], start=False, stop=True)

                    out_sb = pool.tile([N_CMP, N_PTS], dt.float32,
                                       tag="out_sb")
                    nc.vector.tensor_copy(out_sb[:], acc[:])
                    nc.sync.dma_start(partial[:], out_sb[:])

    nc.compile()
    return nc


def _get_compiled():
    global _compiled
    if _compiled is None:
        _compiled = _build_bass()
    return _compiled


def _wrap_idx_full(arr):
    """already-padded (EDGES_PAD,) int array -> wrapped idx layout."""
    a = np.asarray(arr, np.int16)
    return np.tile(a.reshape(-1, 16).T.copy(), (8, 1))


def _wrap_idx(idx_slice):
    """int32 edge-endpoint slice -> (128, EDGES_PAD//16) int16 wrapped layout
    (index i at partition i%16, column i//16; 16-row block replicated 8x)."""
    pad = np.zeros(EDGES_PAD, np.int16)
    pad[:idx_slice.shape[0]] = idx_slice.astype(np.int16)
    arr16 = pad.reshape(EDGES_PAD // 16, 16).T.copy()
    return np.tile(arr16, (8, 1))


def _first_use_table(idx_stream, acts_t):
    """Per-core table whose rows follow the gather stream's first-use
    order, plus the remapped index stream: first occurrences then read
    the table sequentially (DRAM row-buffer friendly); repeats jump back
    to nearby recently-used rows."""
    uniq, fidx = np.unique(idx_stream, return_index=True)
    nodes_in_order = uniq[np.argsort(fidx)]
    pos = np.zeros(N_NODES, np.int32)
    pos[nodes_in_order] = np.arange(len(uniq))
    table = np.zeros((N_NODES, N_PTS), acts_t.dtype)
    table[:len(uniq)] = acts_t[nodes_in_order]
    return table, pos[idx_stream]


def prepare_in_maps(node_activations, binarized_learned, edge_endnode_idx,
                    sort_edges=True, remap_tables=True):
    """Host-side sharding/relayout. Returns (in_maps, K_bias).

    sort_edges: the ab plane is symmetric in the two gathered operands,
    so both the slot order and the per-edge operand orientation are free
    (coefficients follow).  Orient a=min(u,v), b=max(u,v) and sort slots
    by (a >> 9, b): the a-gather walks 256KB row blocks and the b-gather
    gets sorted runs within each block - measured ~5-20% faster than
    unsorted via DRAM row-buffer locality, at zero device cost."""
    act = np.asarray(node_activations)
    W = np.asarray(binarized_learned)
    idx = np.asarray(edge_endnode_idx)

    # fp8 node-activation table, transposed: row v = act[:, v] over 512 pts
    acts_t = np.ascontiguousarray(act.T).astype(np.float32).astype(F8)

    # weight planes over EDGE_TYPES=[0,2,3,5,9]; multilinear coefficients
    P = W.reshape(N_CMP, 5, N_EDGES)
    P1, P2, P3, P4 = P[:, 1], P[:, 2], P[:, 3], P[:, 4]
    K_bias = P1.sum(axis=1, dtype=np.float64).astype(np.float32)  # (64,)
    coeff_a = P3 - P1
    coeff_b = P2 - P1
    coeff_ab = P4 - P3 - P2 + P1

    # node-space fold of the two linear planes: Wnode (N_NODES, N_CMP)
    Wnode = np.zeros((N_NODES, N_CMP), np.float32)
    np.add.at(Wnode, idx[:, 0], coeff_a.T)
    np.add.at(Wnode, idx[:, 1], coeff_b.T)

    # transposed activations for the node matmul: actnd_full[v, n] = act[n, v]
    actnd_full = np.ascontiguousarray(act.T).astype(np.float32)

    in_maps = []
    for s in range(N_CORES):
        sl = slice(s * EDGES_PER_CORE, (s + 1) * EDGES_PER_CORE)
        ia = idx[sl, 0]
        ib = idx[sl, 1]
        cab = coeff_ab[:, sl]
        if sort_edges:
            lo = np.minimum(ia, ib)
            hi = np.maximum(ia, ib)
            perm = np.lexsort((hi, lo >> 9))
            ia, ib, cab = lo[perm], hi[perm], cab[:, perm]
        if remap_tables:
            tab_a, ia = _first_use_table(ia, acts_t)
            tab_b, ib = _first_use_table(ib, acts_t)
        else:
            tab_a = tab_b = acts_t
        cpad = np.zeros((N_CMP, EDGES_PAD), np.float32)
        cpad[:, :EDGES_PER_CORE] = cab
        # lhsT layout: wab[p, j, c] = coeff_ab[c, j*128 + p]
        wab = np.ascontiguousarray(
            cpad.reshape(N_CMP, J_CHUNKS, 128).transpose(2, 1, 0)
        ).astype(F8)

        vsl = slice(s * NODES_PER_CORE, (s + 1) * NODES_PER_CORE)
        wnd_pad = np.zeros((NODES_PAD, N_CMP), np.float32)
        wnd_pad[:NODES_PER_CORE] = Wnode[vsl]
        # split into two exact fp8 planes: W = lo + 16*hi, lo in [-8,8],
        # 16*hi in {0,+-16,+-32,+-48,...} - all e4m3-exact for |W| <= ~400
        hi = np.round(wnd_pad / 16.0)
        lo = wnd_pad - 16.0 * hi
        assert np.abs(hi).max() * 16 <= 448 and np.abs(lo).max() <= 8
        # lhsT layout [128, 2, V_CHUNKS, N_CMP]: [p,h,v,c] = plane_h[v*128+p, c]
        wnd = np.ascontiguousarray(
            np.stack([lo, 16.0 * hi], axis=0)
            .reshape(2, V_CHUNKS, 128, N_CMP).transpose(2, 0, 1, 3)
        ).astype(F8)

        and_pad = np.zeros((NODES_PAD, N_PTS), np.float32)
        and_pad[:NODES_PER_CORE] = actnd_full[vsl]
        actnd = np.ascontiguousarray(
            and_pad.reshape(V_CHUNKS, 128, N_PTS).transpose(1, 0, 2)
        ).astype(F8)

        in_maps.append({
            "acts": tab_a,
            "acts_b": tab_b,
            "idx_a": _wrap_idx(ia),
            "idx_b": _wrap_idx(ib),
            "wp_ab": wab,
            "wp_nd": wnd,
            "acts_nd": actnd,
        })
    return in_maps, K_bias


def postprocess(results, K_bias):
    """Sum per-core partials, add bias, subtract global min."""
    total = np.zeros((N_CMP, N_PTS), np.float32)
    for r in results:
        total += r["partial"]
    energies = total + K_bias[:, None]
    out = energies.T - energies.min()
    return np.ascontiguousarray(out.astype(np.float32))


def kernel(node_activations, binarized_learned, edge_endnode_idx,
           _bass_kwargs=None):
    from concourse.bass_utils import run_bass_kernel_spmd

    nc = _get_compiled()
    in_maps, K_bias = prepare_in_maps(
        node_activations, binarized_learned, edge_endnode_idx)
    res = run_bass_kernel_spmd(nc, in_maps, core_ids=list(range(N_CORES)),
                               **(_bass_kwargs or {}))
    out = postprocess(res.results, K_bias)
    kernel.last_results = res
    return out
